# revision 10
# baseline (speedup 1.0000x reference)
"""FAGCNConv Trainium2 kernel (8 NeuronCores, destination-sharded edges). v3

Algorithm (matches reference up to fp rounding):
    s2b = x @ w2 + b                      (per destination node, local slice)
    sr_e = x[row_e] @ w1                  (per edge, fused DVE mul-reduce on gathered rows)
    sc_e = s2b[col_e]                     (per edge, fused one-hot dot vs broadcast s2 block)
    p_e  = exp(tanh(sr_e + sc_e))         (tanh bounds scores, so softmax max-shift is unneeded)
    acc[v], segsum[v] = sum_{e->v} p_e * x[row_e],  sum_{e->v} p_e   (one-hot matmuls into PSUM)
    out[v] = (1-EPS) * acc[v]/segsum[v] + EPS * x[v]

v3 changes (why): HW probing showed the kernel is bound by dma_gather's
per-descriptor latency (~8.6ns/desc aggregate at 512B single-queue).
  - x table gathered in bf16 (256B elements: ~6.1ns/desc, -30%)
  - gathers striped across 4 SWDGE queues (~1.3x from latency hiding)
  - per-block tile counts (max over cores) instead of global max (-8% descs)
  - one-hot + propagation matmuls in bf16 (PE fp32 4cyc/row -> bf16 1cyc/row)

Sharding: core c owns destinations [c*6250, (c+1)*6250), 49 blocks of 128 dst.
Host sorts edges by (block, row>=32768) and pads each block's lo/hi sections to
the max count over the 8 cores so the SPMD program is identical across cores.
"""

import os
import sys

sys.path.insert(0, "/opt/trn_rl_repo")

import numpy as np
import ml_dtypes

BF16 = np.dtype(ml_dtypes.bfloat16)

N_NODES = 50000
C = 128
EPS = 0.1
NCORES = 8
NLOC = N_NODES // NCORES          # 6250
NBLK = (NLOC + 127) // 128        # 49 (48 full, last has 106 dst)
P = 128
HALF = 32768                      # int16 index limit for dma_gather
DUMMY_COLREL = 200.0
A2_CHUNK = 512
NQ = 4                            # SWDGE queues to stripe gathers over
CE = 256                          # gather element width in bf16: [x(128)|1.0|pad]


def _wrap_idx16(lst):
    """dma_gather index layout: [128, N/16] int16; idx i at [i%16, i//16],
    replicated across the 8 groups of 16 partitions."""
    n = len(lst)
    assert n % 128 == 0
    a16 = np.zeros((16, n // 16), dtype=np.int16)
    a16[np.arange(n) % 16, np.arange(n) // 16] = lst
    return np.tile(a16, (8, 1))


def _prep_shards(edge_index: np.ndarray):
    row_g = edge_index[0].astype(np.int64)
    col_g = edge_index[1].astype(np.int64)
    core_of = col_g // NLOC

    per_core = []
    cnt = np.zeros((NCORES, NBLK, 2), dtype=np.int64)
    for c in range(NCORES):
        m = core_of == c
        r = row_g[m]
        cl = col_g[m] - c * NLOC
        blk = cl // P
        hi = (r >= HALF).astype(np.int64)
        key = blk * 2 + hi
        counts = np.bincount(key, minlength=NBLK * 2)
        cnt[c] = counts.reshape(NBLK, 2)
        per_core.append((r, cl, blk, hi, key, counts))

    cmax = cnt.max(axis=0)                      # [NBLK, 2]
    TBLs = ((cmax[:, 0] + P - 1) // P).astype(int).tolist()
    TBHs = ((cmax[:, 1] + P - 1) // P).astype(int).tolist()
    TBs = [a + b for a, b in zip(TBLs, TBHs)]
    TILES = int(sum(TBs))
    # tile-offset of each block's section
    toff = np.concatenate([[0], np.cumsum(TBs)]).astype(int)

    shards = []
    for c in range(NCORES):
        r, cl, blk, hi, key, counts = per_core[c]
        idx_slot = np.zeros(TILES * P, dtype=np.int64)
        colrel_slot = np.full(TILES * P, DUMMY_COLREL, dtype=np.float32)

        order = np.argsort(key, kind="stable")
        starts = np.zeros(NBLK * 2, dtype=np.int64)
        starts[1:] = np.cumsum(counts)[:-1]
        pos_in_sec = np.arange(len(order)) - starts[key[order]]
        ro, clo, blko, hio = r[order], cl[order], blk[order], hi[order]
        sec_base = (toff[blko] + hio * np.asarray(TBLs)[blko]) * P
        slot = sec_base + pos_in_sec
        idx_slot[slot] = ro - hio * HALF
        colrel_slot[slot] = (clo - blko * P).astype(np.float32)

        idx16_cols = []
        for b in range(NBLK):
            base = toff[b] * P
            mid = base + TBLs[b] * P
            end = base + TBs[b] * P
            idx16_cols.append(_wrap_idx16(idx_slot[base:mid]))
            idx16_cols.append(_wrap_idx16(idx_slot[mid:end]))
        idx16 = np.concatenate(idx16_cols, axis=1)  # [128, TILES*8]
        colrel_T = np.ascontiguousarray(
            colrel_slot.reshape(TILES, P).T
        )  # [128, TILES]
        shards.append(dict(idx16=idx16, colrel_T=colrel_T))
    return tuple(TBLs), tuple(TBHs), shards


def _build_nc(TBLs, TBHs):
    import concourse.bacc as bacc
    import concourse.bass as bass
    import concourse.mybir as mybir
    from concourse.tile import TileContext

    f32 = mybir.dt.float32
    bf16 = mybir.dt.bfloat16
    i16 = mybir.dt.int16
    TBs = [a + b for a, b in zip(TBLs, TBHs)]
    TILES = int(sum(TBs))
    toff = [0]
    for t in TBs:
        toff.append(toff[-1] + t)
    NLOC_PAD = NBLK * P  # 6272

    nc = bacc.Bacc("TRN2", target_bir_lowering=False, num_swdge_queues=NQ)

    xlo_d = nc.dram_tensor("xlo", [HALF, CE], bf16, kind="ExternalInput")
    xhi_d = nc.dram_tensor("xhi", [N_NODES - HALF, CE], bf16, kind="ExternalInput")
    xloc_d = nc.dram_tensor("xloc", [NLOC, C], f32, kind="ExternalInput")
    xlocT_d = nc.dram_tensor("xlocT", [P, NLOC], f32, kind="ExternalInput")
    idx16_d = nc.dram_tensor("idx16", [P, TILES * 8], i16, kind="ExternalInput")
    colrel_d = nc.dram_tensor("colrel", [P, TILES], f32, kind="ExternalInput")
    gw_d = nc.dram_tensor("gate_w", [2 * C, 1], f32, kind="ExternalInput")
    gb_d = nc.dram_tensor("gate_b", [1], f32, kind="ExternalInput")
    iota_d = nc.dram_tensor("iotaf", [P, P], f32, kind="ExternalInput")
    w1b_d = nc.dram_tensor("w1b", [P, P], f32, kind="ExternalInput")
    out_d = nc.dram_tensor("out", [NLOC, C], f32, kind="ExternalOutput")

    s2b_d = nc.dram_tensor("s2b_scratch", [1, NLOC_PAD], f32)

    with TileContext(nc) as tc:
        with (
            tc.tile_pool(name="const", bufs=1) as cpool,
            tc.tile_pool(name="phA", bufs=3) as apool,
            tc.tile_pool(name="phA_ps", bufs=2, space="PSUM") as apsum,
            tc.tile_pool(name="ybuf", bufs=3) as ypool,
            tc.tile_pool(name="blk", bufs=3) as bpool,
            tc.tile_pool(name="small", bufs=4) as spool,
            tc.tile_pool(name="oh", bufs=22) as ohpool,
            tc.tile_pool(name="acc_ps", bufs=2, space="PSUM") as bpsum,
        ):
            iotaf = cpool.tile([P, P], f32)
            nc.sync.dma_start(iotaf[:], iota_d[:])
            iotab = cpool.tile([P, P], bf16)
            nc.vector.tensor_scalar(
                iotab[:], iotaf[:], 1.0, None, op0=mybir.AluOpType.mult
            )
            w1b = cpool.tile([P, P], f32)
            nc.sync.dma_start(w1b[:], w1b_d[:])
            w2 = cpool.tile([P, 1], f32)
            nc.sync.dma_start(w2[:], gw_d[C : 2 * C, 0:1])
            btile = cpool.tile([1, 1], f32)
            nc.sync.dma_start(btile[:], gb_d[:, None])
            ones_col = cpool.tile([P, 1], bf16)
            nc.vector.memset(ones_col[:], 1.0)
            zpad = cpool.tile([1, NLOC_PAD - NLOC], f32)
            nc.vector.memset(zpad[:], 0.0)
            nc.sync.dma_start(s2b_d[0:1, NLOC:NLOC_PAD], zpad[:])
            s2all = cpool.tile([P, NLOC_PAD], f32)

            # ---- Phase A: s2b[v] = xloc[v] @ w2 + b (local nodes) ----
            nck = (NLOC + A2_CHUNK - 1) // A2_CHUNK
            for k in range(nck):
                a = k * A2_CHUNK
                n = min(A2_CHUNK, NLOC - a)
                xck = apool.tile([P, A2_CHUNK], f32, tag="xck")
                nc.sync.dma_start(xck[:, :n], xlocT_d[:, a : a + n])
                ps = apsum.tile([1, A2_CHUNK], f32, tag="s2ps")
                nc.tensor.matmul(
                    out=ps[:, :n], lhsT=w2[:], rhs=xck[:, :n], start=True, stop=True
                )
                s2sb = apool.tile([1, A2_CHUNK], f32, tag="s2sb")
                nc.scalar.activation(
                    s2sb[:, :n],
                    ps[:, :n],
                    mybir.ActivationFunctionType.Identity,
                    bias=btile[:],
                    scale=1.0,
                )
                nc.sync.dma_start(s2b_d[0:1, a : a + n], s2sb[:, :n])

            # one broadcast of the whole s2 row into SBUF (replaces 49
            # per-block 64KB broadcast DMAs)
            nc.sync.dma_start(
                s2all[:], s2b_d[0:1, :].to_broadcast((P, NLOC_PAD))
            )

            # ---- Phase B ----
            nblk_run = int(os.environ.get("KERNEL_NBLK", NBLK))
            skips = set(os.environ.get("KERNEL_SKIP", "").split(","))
            qctr = 0
            for b in range(nblk_run):
                TBL, TBH = TBLs[b], TBHs[b]
                TB = TBL + TBH
                nd = min(P, NLOC - b * P)
                t0 = toff[b]

                colrel_blk = spool.tile([P, TB], f32, tag="colrel")
                nc.sync.dma_start(colrel_blk[:], colrel_d[:, t0 : t0 + TB])
                idxlo = spool.tile([P, TBL * 8], i16, tag="idxlo")
                nc.sync.dma_start(
                    idxlo[:], idx16_d[:, t0 * 8 : t0 * 8 + TBL * 8]
                )
                idxhi = spool.tile([P, TBH * 8], i16, tag="idxhi")
                nc.sync.dma_start(
                    idxhi[:], idx16_d[:, t0 * 8 + TBL * 8 : (t0 + TB) * 8]
                )
                s2bc = s2all[:, b * P : (b + 1) * P]

                Y_blk = ypool.tile([P, TB * CE], bf16, tag="Y")
                if "gather" in skips:
                    nc.vector.memset(Y_blk[:], 0.5)
                else:
                    nc.gpsimd.dma_gather(
                        Y_blk[:, : TBL * CE].rearrange("p (t c) -> p t c", c=CE),
                        xlo_d[:],
                        idxlo[:],
                        TBL * P,
                        TBL * P,
                        CE,
                        single_packet=False,
                        queue_num=qctr % NQ,
                    )
                    qctr += 1
                    nc.gpsimd.dma_gather(
                        Y_blk[:, TBL * CE :].rearrange("p (t c) -> p t c", c=CE),
                        xhi_d[:],
                        idxhi[:],
                        TBH * P,
                        TBH * P,
                        CE,
                        single_packet=False,
                        queue_num=qctr % NQ,
                    )
                    qctr += 1

                sr_blk = spool.tile([P, TB], f32, tag="sr")
                sc_blk = spool.tile([P, TB], f32, tag="sc")
                if "stt" in skips:
                    nc.vector.memset(sr_blk[:], 0.1)
                    nc.vector.memset(sc_blk[:], 0.1)
                for t in range([0, TB]["stt" not in skips]):
                    scr1 = ohpool.tile([P, P], bf16, tag="scr1")
                    nc.vector.scalar_tensor_tensor(
                        out=scr1[:],
                        in0=Y_blk[:, t * CE : t * CE + C],
                        scalar=1.0,
                        in1=w1b[:],
                        op0=mybir.AluOpType.mult,
                        op1=mybir.AluOpType.mult,
                        accum_out=sr_blk[:, t : t + 1],
                    )
                    scr2 = ohpool.tile([P, P], bf16, tag="scr2")
                    nc.vector.scalar_tensor_tensor(
                        out=scr2[:],
                        in0=iotaf[:],
                        scalar=colrel_blk[:, t : t + 1],
                        in1=s2bc,
                        op0=mybir.AluOpType.is_equal,
                        op1=mybir.AluOpType.mult,
                        accum_out=sc_blk[:, t : t + 1],
                    )

                u_blk = spool.tile([P, TB], f32, tag="u")
                nc.vector.tensor_tensor(
                    out=u_blk[:],
                    in0=sr_blk[:],
                    in1=sc_blk[:],
                    op=mybir.AluOpType.add,
                )
                th_blk = spool.tile([P, TB], f32, tag="th")
                nc.scalar.activation(
                    th_blk[:], u_blk[:], mybir.ActivationFunctionType.Tanh
                )
                p_blk = spool.tile([P, TB], f32, tag="p")
                nc.scalar.activation(
                    p_blk[:], th_blk[:], mybir.ActivationFunctionType.Exp
                )

                # col 128 of each gathered element is a host-written 1.0, so a
                # single [P, C+1] matmul accumulates propagation AND segsum.
                acc_x = bpsum.tile([P, C + 1], f32, tag="accx")
                for t in range([0, TB]["mm" not in skips]):
                    onehot = ohpool.tile([P, P], bf16, tag="onehot")
                    nc.vector.tensor_scalar(
                        onehot[:],
                        iotab[:],
                        colrel_blk[:, t : t + 1],
                        p_blk[:, t : t + 1],
                        op0=mybir.AluOpType.is_equal,
                        op1=mybir.AluOpType.mult,
                    )
                    nc.tensor.matmul(
                        out=acc_x[:],
                        lhsT=onehot[:],
                        rhs=Y_blk[:, t * CE : t * CE + C + 1],
                        start=(t == 0),
                        stop=(t == TB - 1),
                    )

                if "mm" in skips:
                    nc.tensor.matmul(out=acc_x[:, :C], lhsT=iotab[:], rhs=iotab[:], start=True, stop=False)
                    nc.tensor.matmul(out=acc_x[:, C : C + 1], lhsT=iotab[:], rhs=ones_col[:], start=False, stop=True)
                segsum = spool.tile([P, 1], f32, tag="segsum")
                nc.vector.tensor_scalar(
                    segsum[:], acc_x[:, C : C + 1], 1e-30, None, op0=mybir.AluOpType.add
                )
                inv = spool.tile([P, 1], f32, tag="inv")
                nc.vector.reciprocal(inv[:], segsum[:])
                inv9 = spool.tile([P, 1], f32, tag="inv9")
                nc.scalar.mul(inv9[:], inv[:], 1.0 - EPS)

                xblk = bpool.tile([P, C], f32, tag="xblk")
                nc.sync.dma_start(xblk[:nd, :], xloc_d[b * P : b * P + nd, :])
                o1 = bpool.tile([P, C], f32, tag="o1")
                nc.vector.tensor_scalar(
                    o1[:], acc_x[:, :C], inv9[:], None, op0=mybir.AluOpType.mult
                )
                oblk = bpool.tile([P, C], f32, tag="oblk")
                nc.vector.scalar_tensor_tensor(
                    oblk[:nd, :],
                    xblk[:nd, :],
                    EPS,
                    o1[:nd, :],
                    op0=mybir.AluOpType.mult,
                    op1=mybir.AluOpType.add,
                )
                nc.sync.dma_start(out_d[b * P : b * P + nd, :], oblk[:nd, :])

    nc.finalize()
    return nc


_CACHE = {}


def _get_nc(TBLs, TBHs):
    key = (TBLs, TBHs)
    if key not in _CACHE:
        _CACHE[key] = _build_nc(TBLs, TBHs)
    return _CACHE[key]


def _make_in_maps(x, edge_index, gate_w, gate_b):
    TBLs, TBHs, shards = _prep_shards(edge_index)
    iotaf = np.broadcast_to(np.arange(P, dtype=np.float32)[None, :], (P, P)).copy()
    w1b = np.broadcast_to(gate_w[:C, 0][None, :], (P, C)).copy()
    xaug = np.zeros((N_NODES, CE), dtype=BF16)
    xaug[:, :C] = x.astype(BF16)
    xaug[:, C] = np.float32(1.0)
    xlo = np.ascontiguousarray(xaug[:HALF])
    xhi = np.ascontiguousarray(xaug[HALF:])
    in_maps = []
    for c in range(NCORES):
        xloc = np.ascontiguousarray(x[c * NLOC : (c + 1) * NLOC])
        in_maps.append(
            {
                "xlo": xlo,
                "xhi": xhi,
                "xloc": xloc,
                "xlocT": np.ascontiguousarray(xloc.T),
                "idx16": shards[c]["idx16"],
                "colrel": shards[c]["colrel_T"],
                "gate_w": gate_w,
                "gate_b": gate_b,
                "iotaf": iotaf,
                "w1b": w1b,
            }
        )
    return TBLs, TBHs, in_maps


def kernel(x, edge_index, gate_w, gate_b):
    from concourse.bass_utils import run_bass_kernel_spmd

    x = np.asarray(x, dtype=np.float32)
    edge_index = np.asarray(edge_index, dtype=np.int32)
    gate_w = np.asarray(gate_w, dtype=np.float32)
    gate_b = np.asarray(gate_b, dtype=np.float32)

    TBLs, TBHs, in_maps = _make_in_maps(x, edge_index, gate_w, gate_b)
    nc = _get_nc(TBLs, TBHs)

    res = run_bass_kernel_spmd(nc, in_maps, core_ids=list(range(NCORES)))
    out = np.concatenate([res.results[c]["out"] for c in range(NCORES)], axis=0)
    return out


def time_kernel(inputs, iters=32, iters_lo=2, reps=4):
    """Estimate per-execution HW time: async-dispatch M executions of one jitted
    single-exec program (device executions serialize per core); per-exec time =
    (T(M_hi) - T(M_lo)) / (M_hi - M_lo), min over reps."""
    import time as _time

    import jax
    import concourse.mybir as mybir
    from concourse import bass2jax as b2j

    x = np.asarray(inputs["x"], dtype=np.float32)
    edge_index = np.asarray(inputs["edge_index"], dtype=np.int32)
    gate_w = np.asarray(inputs["gate_w"], dtype=np.float32)
    gate_b = np.asarray(inputs["gate_b"], dtype=np.float32)

    TBLs, TBHs, in_maps = _make_in_maps(x, edge_index, gate_w, gate_b)
    nc = _get_nc(TBLs, TBHs)
    b2j.install_neuronx_cc_hook()

    partition_name = nc.partition_id_tensor.name if nc.partition_id_tensor else None
    in_names, out_names, out_avals, zero_outs = [], [], [], []
    for alloc in nc.m.functions[0].allocations:
        if not isinstance(alloc, mybir.MemoryLocationSet):
            continue
        name = alloc.memorylocations[0].name
        if alloc.kind == "ExternalInput":
            if name != partition_name:
                in_names.append(name)
        elif alloc.kind == "ExternalOutput":
            shape = tuple(alloc.tensor_shape)
            dtype = mybir.dt.np(alloc.dtype)
            out_names.append(name)
            out_avals.append(jax.core.ShapedArray(shape, dtype))
            zero_outs.append(np.zeros(shape, dtype))
    n_params = len(in_names)
    all_in_names = in_names + out_names

    def _body(*args):
        operands = list(args)
        if partition_name is not None:
            operands.append(b2j.partition_id_tensor())
        return tuple(
            b2j._bass_exec_p.bind(
                *operands,
                out_avals=tuple(out_avals),
                in_names=tuple(
                    all_in_names + ([partition_name] if partition_name else [])
                ),
                out_names=tuple(out_names),
                lowering_input_output_aliases=(),
                sim_require_finite=True,
                sim_require_nnan=True,
                nc=nc,
            )
        )

    devices = jax.devices()[:NCORES]
    mesh = b2j.Mesh(np.asarray(devices), ("core",))
    in_specs = (b2j.PartitionSpec("core",),) * (n_params + len(out_names))
    out_specs = (b2j.PartitionSpec("core",),) * len(out_names)
    fn = jax.jit(
        b2j.shard_map(
            _body, mesh=mesh, in_specs=in_specs, out_specs=out_specs, check_rep=False
        ),
        keep_unused=True,
    )

    per_core = [[np.asarray(m[name]) for name in in_names] for m in in_maps]
    concat_in = [
        np.concatenate([per_core[c][i] for c in range(NCORES)], axis=0)
        for i in range(n_params)
    ]
    concat_zeros = [
        np.zeros((NCORES * z.shape[0], *z.shape[1:]), z.dtype) for z in zero_outs
    ]

    from jax.sharding import NamedSharding

    sh = NamedSharding(mesh, b2j.PartitionSpec("core"))
    dev_in = [jax.device_put(a, sh) for a in concat_in]
    dev_zero = [jax.device_put(a, sh) for a in concat_zeros]

    jax.block_until_ready(fn(*dev_in, *dev_zero))
    jax.block_until_ready(fn(*dev_in, *dev_zero))

    best = None
    for _ in range(reps):
        t0 = _time.perf_counter()
        rs = [fn(*dev_in, *dev_zero) for _ in range(iters)]
        jax.block_until_ready(rs)
        t_hi = _time.perf_counter() - t0
        del rs
        t0 = _time.perf_counter()
        rs = [fn(*dev_in, *dev_zero) for _ in range(iters_lo)]
        jax.block_until_ready(rs)
        t_lo = _time.perf_counter() - t0
        del rs
        per_exec = (t_hi - t_lo) / (iters - iters_lo)
        print(
            f"  t({iters})={t_hi*1e3:.2f}ms t({iters_lo})={t_lo*1e3:.2f}ms "
            f"per_exec={per_exec*1e6:.1f}us"
        )
        if best is None or per_exec < best:
            best = per_exec
    return best * 1e9


# revision 11
# speedup vs baseline: 1.3282x; 1.3282x over previous
"""FAGCNConv Trainium2 kernel (8 NeuronCores, destination-sharded edges). v3

Algorithm (matches reference up to fp rounding):
    s2b = x @ w2 + b                      (per destination node, local slice)
    sr_e = x[row_e] @ w1                  (per edge, fused DVE mul-reduce on gathered rows)
    sc_e = s2b[col_e]                     (per edge, fused one-hot dot vs broadcast s2 block)
    p_e  = exp(tanh(sr_e + sc_e))         (tanh bounds scores, so softmax max-shift is unneeded)
    acc[v], segsum[v] = sum_{e->v} p_e * x[row_e],  sum_{e->v} p_e   (one-hot matmuls into PSUM)
    out[v] = (1-EPS) * acc[v]/segsum[v] + EPS * x[v]

v3 changes (why): HW probing showed the kernel is bound by dma_gather's
per-descriptor latency (~8.6ns/desc aggregate at 512B single-queue).
  - x table gathered in bf16 (256B elements: ~6.1ns/desc, -30%)
  - gathers striped across 4 SWDGE queues (~1.3x from latency hiding)
  - per-block tile counts (max over cores) instead of global max (-8% descs)
  - one-hot + propagation matmuls in bf16 (PE fp32 4cyc/row -> bf16 1cyc/row)

Sharding: core c owns destinations [c*6250, (c+1)*6250), 49 blocks of 128 dst.
Host sorts edges by (block, row>=32768) and pads each block's lo/hi sections to
the max count over the 8 cores so the SPMD program is identical across cores.
"""

import os
import sys

sys.path.insert(0, "/opt/trn_rl_repo")

import numpy as np
import ml_dtypes

BF16 = np.dtype(ml_dtypes.bfloat16)

N_NODES = 50000
C = 128
EPS = 0.1
NCORES = 8
NLOC = N_NODES // NCORES          # 6250
NBLK = (NLOC + 127) // 128        # 49 (48 full, last has 106 dst)
P = 128
HALF = 32768                      # int16 index limit for dma_gather
DUMMY_COLREL = 200.0
A2_CHUNK = 512
NQ = 4                            # SWDGE queues to stripe gathers over
CE = 256                          # gather element width in bf16: [x(128)|1.0|pad]


def _wrap_idx16(lst):
    """dma_gather index layout: [128, N/16] int16; idx i at [i%16, i//16],
    replicated across the 8 groups of 16 partitions."""
    n = len(lst)
    assert n % 128 == 0
    a16 = np.zeros((16, n // 16), dtype=np.int16)
    a16[np.arange(n) % 16, np.arange(n) // 16] = lst
    return np.tile(a16, (8, 1))


def _prep_shards(edge_index: np.ndarray):
    row_g = edge_index[0].astype(np.int64)
    col_g = edge_index[1].astype(np.int64)
    core_of = col_g // NLOC

    per_core = []
    cnt = np.zeros((NCORES, NBLK, 2), dtype=np.int64)
    for c in range(NCORES):
        m = core_of == c
        r = row_g[m]
        cl = col_g[m] - c * NLOC
        blk = cl // P
        hi = (r >= HALF).astype(np.int64)
        key = blk * 2 + hi
        counts = np.bincount(key, minlength=NBLK * 2)
        cnt[c] = counts.reshape(NBLK, 2)
        per_core.append((r, cl, blk, hi, key, counts))

    cmax = cnt.max(axis=0)                      # [NBLK, 2]
    TBLs = ((cmax[:, 0] + P - 1) // P).astype(int).tolist()
    TBHs = ((cmax[:, 1] + P - 1) // P).astype(int).tolist()
    TBs = [a + b for a, b in zip(TBLs, TBHs)]
    TILES = int(sum(TBs))
    # tile-offset of each block's section
    toff = np.concatenate([[0], np.cumsum(TBs)]).astype(int)

    shards = []
    for c in range(NCORES):
        r, cl, blk, hi, key, counts = per_core[c]
        idx_slot = np.zeros(TILES * P, dtype=np.int64)
        colrel_slot = np.full(TILES * P, DUMMY_COLREL, dtype=np.float32)

        order = np.argsort(key, kind="stable")
        starts = np.zeros(NBLK * 2, dtype=np.int64)
        starts[1:] = np.cumsum(counts)[:-1]
        pos_in_sec = np.arange(len(order)) - starts[key[order]]
        ro, clo, blko, hio = r[order], cl[order], blk[order], hi[order]
        sec_base = (toff[blko] + hio * np.asarray(TBLs)[blko]) * P
        slot = sec_base + pos_in_sec
        idx_slot[slot] = ro - hio * HALF
        colrel_slot[slot] = (clo - blko * P).astype(np.float32)

        idx16_cols = []
        for b in range(NBLK):
            base = toff[b] * P
            mid = base + TBLs[b] * P
            end = base + TBs[b] * P
            idx16_cols.append(_wrap_idx16(idx_slot[base:mid]))
            idx16_cols.append(_wrap_idx16(idx_slot[mid:end]))
        idx16 = np.concatenate(idx16_cols, axis=1)  # [128, TILES*8]
        colrel_T = np.ascontiguousarray(
            colrel_slot.reshape(TILES, P).T
        )  # [128, TILES]
        shards.append(dict(idx16=idx16, colrel_T=colrel_T))
    return tuple(TBLs), tuple(TBHs), shards


def _build_nc(TBLs, TBHs):
    import concourse.bacc as bacc
    import concourse.bass as bass
    import concourse.mybir as mybir
    from concourse.tile import TileContext

    f32 = mybir.dt.float32
    bf16 = mybir.dt.bfloat16
    i16 = mybir.dt.int16
    TBs = [a + b for a, b in zip(TBLs, TBHs)]
    TILES = int(sum(TBs))
    toff = [0]
    for t in TBs:
        toff.append(toff[-1] + t)
    NLOC_PAD = NBLK * P  # 6272

    nc = bacc.Bacc("TRN2", target_bir_lowering=False, num_swdge_queues=NQ)

    xlo_d = nc.dram_tensor("xlo", [HALF, CE], bf16, kind="ExternalInput")
    xhi_d = nc.dram_tensor("xhi", [N_NODES - HALF, CE], bf16, kind="ExternalInput")
    xloc_d = nc.dram_tensor("xloc", [NLOC, C], f32, kind="ExternalInput")
    xlocT_d = nc.dram_tensor("xlocT", [P, NLOC], f32, kind="ExternalInput")
    idx16_d = nc.dram_tensor("idx16", [P, TILES * 8], i16, kind="ExternalInput")
    colrel_d = nc.dram_tensor("colrel", [P, TILES], f32, kind="ExternalInput")
    gw_d = nc.dram_tensor("gate_w", [2 * C, 1], f32, kind="ExternalInput")
    gb_d = nc.dram_tensor("gate_b", [1], f32, kind="ExternalInput")
    iota_d = nc.dram_tensor("iotaf", [P, P], f32, kind="ExternalInput")
    w1b_d = nc.dram_tensor("w1b", [P, P], f32, kind="ExternalInput")
    out_d = nc.dram_tensor("out", [NLOC, C], f32, kind="ExternalOutput")

    s2b_d = nc.dram_tensor("s2b_scratch", [1, NLOC_PAD], f32)

    with TileContext(nc) as tc:
        with (
            tc.tile_pool(name="const", bufs=1) as cpool,
            tc.tile_pool(name="phA", bufs=3) as apool,
            tc.tile_pool(name="phA_ps", bufs=2, space="PSUM") as apsum,
            tc.tile_pool(name="ybuf", bufs=4) as ypool,
            tc.tile_pool(name="blk", bufs=4) as bpool,
            tc.tile_pool(name="small", bufs=6) as spool,
            tc.tile_pool(name="oh", bufs=22) as ohpool,
            tc.tile_pool(name="acc_ps", bufs=2, space="PSUM") as bpsum,
        ):
            iotaf = cpool.tile([P, P], f32)
            nc.sync.dma_start(iotaf[:], iota_d[:])
            iotab = cpool.tile([P, P], bf16)
            nc.vector.tensor_scalar(
                iotab[:], iotaf[:], 1.0, None, op0=mybir.AluOpType.mult
            )
            w1b = cpool.tile([P, P], f32)
            nc.sync.dma_start(w1b[:], w1b_d[:])
            w2 = cpool.tile([P, 1], f32)
            nc.sync.dma_start(w2[:], gw_d[C : 2 * C, 0:1])
            btile = cpool.tile([1, 1], f32)
            nc.sync.dma_start(btile[:], gb_d[:, None])
            ones_col = cpool.tile([P, 1], bf16)
            nc.vector.memset(ones_col[:], 1.0)
            zpad = cpool.tile([1, NLOC_PAD - NLOC], f32)
            nc.vector.memset(zpad[:], 0.0)
            nc.sync.dma_start(s2b_d[0:1, NLOC:NLOC_PAD], zpad[:])
            s2all = cpool.tile([P, NLOC_PAD], f32)

            # ---- Phase A: s2b[v] = xloc[v] @ w2 + b (local nodes) ----
            nck = (NLOC + A2_CHUNK - 1) // A2_CHUNK
            for k in range(nck):
                a = k * A2_CHUNK
                n = min(A2_CHUNK, NLOC - a)
                xck = apool.tile([P, A2_CHUNK], f32, tag="xck")
                nc.sync.dma_start(xck[:, :n], xlocT_d[:, a : a + n])
                ps = apsum.tile([1, A2_CHUNK], f32, tag="s2ps")
                nc.tensor.matmul(
                    out=ps[:, :n], lhsT=w2[:], rhs=xck[:, :n], start=True, stop=True
                )
                s2sb = apool.tile([1, A2_CHUNK], f32, tag="s2sb")
                nc.scalar.activation(
                    s2sb[:, :n],
                    ps[:, :n],
                    mybir.ActivationFunctionType.Identity,
                    bias=btile[:],
                    scale=1.0,
                )
                nc.sync.dma_start(s2b_d[0:1, a : a + n], s2sb[:, :n])

            # one broadcast of the whole s2 row into SBUF (replaces 49
            # per-block 64KB broadcast DMAs)
            nc.sync.dma_start(
                s2all[:], s2b_d[0:1, :].to_broadcast((P, NLOC_PAD))
            )

            # ---- Phase B ----
            nblk_run = int(os.environ.get("KERNEL_NBLK", NBLK))
            skips = set(os.environ.get("KERNEL_SKIP", "").split(","))
            qctr = 0
            for b in range(nblk_run):
                TBL, TBH = TBLs[b], TBHs[b]
                TB = TBL + TBH
                nd = min(P, NLOC - b * P)
                t0 = toff[b]

                colrel_blk = spool.tile([P, TB], f32, tag="colrel")
                nc.sync.dma_start(colrel_blk[:], colrel_d[:, t0 : t0 + TB])
                idxlo = spool.tile([P, TBL * 8], i16, tag="idxlo")
                nc.sync.dma_start(
                    idxlo[:], idx16_d[:, t0 * 8 : t0 * 8 + TBL * 8]
                )
                idxhi = spool.tile([P, TBH * 8], i16, tag="idxhi")
                nc.sync.dma_start(
                    idxhi[:], idx16_d[:, t0 * 8 + TBL * 8 : (t0 + TB) * 8]
                )
                s2bc = s2all[:, b * P : (b + 1) * P]

                Y_blk = ypool.tile([P, TB * CE], bf16, tag="Y")
                if "gather" in skips:
                    nc.vector.memset(Y_blk[:], 0.5)
                else:
                    nc.gpsimd.dma_gather(
                        Y_blk[:, : TBL * CE].rearrange("p (t c) -> p t c", c=CE),
                        xlo_d[:],
                        idxlo[:],
                        TBL * P,
                        TBL * P,
                        CE,
                        single_packet=False,
                        queue_num=qctr % NQ,
                    )
                    qctr += 1
                    nc.gpsimd.dma_gather(
                        Y_blk[:, TBL * CE :].rearrange("p (t c) -> p t c", c=CE),
                        xhi_d[:],
                        idxhi[:],
                        TBH * P,
                        TBH * P,
                        CE,
                        single_packet=False,
                        queue_num=qctr % NQ,
                    )
                    qctr += 1

                sr_blk = spool.tile([P, TB], f32, tag="sr")
                sc_blk = spool.tile([P, TB], f32, tag="sc")
                if "stt" in skips:
                    nc.vector.memset(sr_blk[:], 0.1)
                    nc.vector.memset(sc_blk[:], 0.1)
                for t in range([0, TB]["stt" not in skips]):
                    scr1 = ohpool.tile([P, P], bf16, tag="scr1")
                    nc.vector.scalar_tensor_tensor(
                        out=scr1[:],
                        in0=Y_blk[:, t * CE : t * CE + C],
                        scalar=1.0,
                        in1=w1b[:],
                        op0=mybir.AluOpType.mult,
                        op1=mybir.AluOpType.mult,
                        accum_out=sr_blk[:, t : t + 1],
                    )
                    scr2 = ohpool.tile([P, P], bf16, tag="scr2")
                    nc.vector.scalar_tensor_tensor(
                        out=scr2[:],
                        in0=iotaf[:],
                        scalar=colrel_blk[:, t : t + 1],
                        in1=s2bc,
                        op0=mybir.AluOpType.is_equal,
                        op1=mybir.AluOpType.mult,
                        accum_out=sc_blk[:, t : t + 1],
                    )

                u_blk = spool.tile([P, TB], f32, tag="u")
                nc.vector.tensor_tensor(
                    out=u_blk[:],
                    in0=sr_blk[:],
                    in1=sc_blk[:],
                    op=mybir.AluOpType.add,
                )
                th_blk = spool.tile([P, TB], f32, tag="th")
                nc.scalar.activation(
                    th_blk[:], u_blk[:], mybir.ActivationFunctionType.Tanh
                )
                p_blk = spool.tile([P, TB], f32, tag="p")
                nc.scalar.activation(
                    p_blk[:], th_blk[:], mybir.ActivationFunctionType.Exp
                )

                # col 128 of each gathered element is a host-written 1.0, so a
                # single [P, C+1] matmul accumulates propagation AND segsum.
                acc_x = bpsum.tile([P, C + 1], f32, tag="accx")
                for t in range([0, TB]["mm" not in skips]):
                    onehot = ohpool.tile([P, P], bf16, tag="onehot")
                    nc.vector.tensor_scalar(
                        onehot[:],
                        iotab[:],
                        colrel_blk[:, t : t + 1],
                        p_blk[:, t : t + 1],
                        op0=mybir.AluOpType.is_equal,
                        op1=mybir.AluOpType.mult,
                    )
                    nc.tensor.matmul(
                        out=acc_x[:],
                        lhsT=onehot[:],
                        rhs=Y_blk[:, t * CE : t * CE + C + 1],
                        start=(t == 0),
                        stop=(t == TB - 1),
                    )

                if "mm" in skips:
                    nc.tensor.matmul(out=acc_x[:, :C], lhsT=iotab[:], rhs=iotab[:], start=True, stop=False)
                    nc.tensor.matmul(out=acc_x[:, C : C + 1], lhsT=iotab[:], rhs=ones_col[:], start=False, stop=True)
                segsum = spool.tile([P, 1], f32, tag="segsum")
                nc.vector.tensor_scalar(
                    segsum[:], acc_x[:, C : C + 1], 1e-30, None, op0=mybir.AluOpType.add
                )
                inv = spool.tile([P, 1], f32, tag="inv")
                nc.vector.reciprocal(inv[:], segsum[:])
                inv9 = spool.tile([P, 1], f32, tag="inv9")
                nc.scalar.mul(inv9[:], inv[:], 1.0 - EPS)

                xblk = bpool.tile([P, C], f32, tag="xblk")
                nc.sync.dma_start(xblk[:nd, :], xloc_d[b * P : b * P + nd, :])
                o1 = bpool.tile([P, C], f32, tag="o1")
                nc.vector.tensor_scalar(
                    o1[:], acc_x[:, :C], inv9[:], None, op0=mybir.AluOpType.mult
                )
                oblk = bpool.tile([P, C], f32, tag="oblk")
                nc.vector.scalar_tensor_tensor(
                    oblk[:nd, :],
                    xblk[:nd, :],
                    EPS,
                    o1[:nd, :],
                    op0=mybir.AluOpType.mult,
                    op1=mybir.AluOpType.add,
                )
                nc.sync.dma_start(out_d[b * P : b * P + nd, :], oblk[:nd, :])

    nc.finalize()
    return nc


_CACHE = {}


def _get_nc(TBLs, TBHs):
    key = (TBLs, TBHs)
    if key not in _CACHE:
        _CACHE[key] = _build_nc(TBLs, TBHs)
    return _CACHE[key]


def _make_in_maps(x, edge_index, gate_w, gate_b):
    TBLs, TBHs, shards = _prep_shards(edge_index)
    iotaf = np.broadcast_to(np.arange(P, dtype=np.float32)[None, :], (P, P)).copy()
    w1b = np.broadcast_to(gate_w[:C, 0][None, :], (P, C)).copy()
    xaug = np.zeros((N_NODES, CE), dtype=BF16)
    xaug[:, :C] = x.astype(BF16)
    xaug[:, C] = np.float32(1.0)
    xlo = np.ascontiguousarray(xaug[:HALF])
    xhi = np.ascontiguousarray(xaug[HALF:])
    in_maps = []
    for c in range(NCORES):
        xloc = np.ascontiguousarray(x[c * NLOC : (c + 1) * NLOC])
        in_maps.append(
            {
                "xlo": xlo,
                "xhi": xhi,
                "xloc": xloc,
                "xlocT": np.ascontiguousarray(xloc.T),
                "idx16": shards[c]["idx16"],
                "colrel": shards[c]["colrel_T"],
                "gate_w": gate_w,
                "gate_b": gate_b,
                "iotaf": iotaf,
                "w1b": w1b,
            }
        )
    return TBLs, TBHs, in_maps


def kernel(x, edge_index, gate_w, gate_b):
    from concourse.bass_utils import run_bass_kernel_spmd

    x = np.asarray(x, dtype=np.float32)
    edge_index = np.asarray(edge_index, dtype=np.int32)
    gate_w = np.asarray(gate_w, dtype=np.float32)
    gate_b = np.asarray(gate_b, dtype=np.float32)

    TBLs, TBHs, in_maps = _make_in_maps(x, edge_index, gate_w, gate_b)
    nc = _get_nc(TBLs, TBHs)

    res = run_bass_kernel_spmd(nc, in_maps, core_ids=list(range(NCORES)))
    out = np.concatenate([res.results[c]["out"] for c in range(NCORES)], axis=0)
    return out


def time_kernel(inputs, iters=32, iters_lo=2, reps=4):
    """Estimate per-execution HW time: async-dispatch M executions of one jitted
    single-exec program (device executions serialize per core); per-exec time =
    (T(M_hi) - T(M_lo)) / (M_hi - M_lo), min over reps."""
    import time as _time

    import jax
    import concourse.mybir as mybir
    from concourse import bass2jax as b2j

    x = np.asarray(inputs["x"], dtype=np.float32)
    edge_index = np.asarray(inputs["edge_index"], dtype=np.int32)
    gate_w = np.asarray(inputs["gate_w"], dtype=np.float32)
    gate_b = np.asarray(inputs["gate_b"], dtype=np.float32)

    TBLs, TBHs, in_maps = _make_in_maps(x, edge_index, gate_w, gate_b)
    nc = _get_nc(TBLs, TBHs)
    b2j.install_neuronx_cc_hook()

    partition_name = nc.partition_id_tensor.name if nc.partition_id_tensor else None
    in_names, out_names, out_avals, zero_outs = [], [], [], []
    for alloc in nc.m.functions[0].allocations:
        if not isinstance(alloc, mybir.MemoryLocationSet):
            continue
        name = alloc.memorylocations[0].name
        if alloc.kind == "ExternalInput":
            if name != partition_name:
                in_names.append(name)
        elif alloc.kind == "ExternalOutput":
            shape = tuple(alloc.tensor_shape)
            dtype = mybir.dt.np(alloc.dtype)
            out_names.append(name)
            out_avals.append(jax.core.ShapedArray(shape, dtype))
            zero_outs.append(np.zeros(shape, dtype))
    n_params = len(in_names)
    all_in_names = in_names + out_names

    def _body(*args):
        operands = list(args)
        if partition_name is not None:
            operands.append(b2j.partition_id_tensor())
        return tuple(
            b2j._bass_exec_p.bind(
                *operands,
                out_avals=tuple(out_avals),
                in_names=tuple(
                    all_in_names + ([partition_name] if partition_name else [])
                ),
                out_names=tuple(out_names),
                lowering_input_output_aliases=(),
                sim_require_finite=True,
                sim_require_nnan=True,
                nc=nc,
            )
        )

    devices = jax.devices()[:NCORES]
    mesh = b2j.Mesh(np.asarray(devices), ("core",))
    in_specs = (b2j.PartitionSpec("core",),) * (n_params + len(out_names))
    out_specs = (b2j.PartitionSpec("core",),) * len(out_names)
    fn = jax.jit(
        b2j.shard_map(
            _body, mesh=mesh, in_specs=in_specs, out_specs=out_specs, check_rep=False
        ),
        keep_unused=True,
    )

    per_core = [[np.asarray(m[name]) for name in in_names] for m in in_maps]
    concat_in = [
        np.concatenate([per_core[c][i] for c in range(NCORES)], axis=0)
        for i in range(n_params)
    ]
    concat_zeros = [
        np.zeros((NCORES * z.shape[0], *z.shape[1:]), z.dtype) for z in zero_outs
    ]

    from jax.sharding import NamedSharding

    sh = NamedSharding(mesh, b2j.PartitionSpec("core"))
    dev_in = [jax.device_put(a, sh) for a in concat_in]
    dev_zero = [jax.device_put(a, sh) for a in concat_zeros]

    jax.block_until_ready(fn(*dev_in, *dev_zero))
    jax.block_until_ready(fn(*dev_in, *dev_zero))

    best = None
    for _ in range(reps):
        t0 = _time.perf_counter()
        rs = [fn(*dev_in, *dev_zero) for _ in range(iters)]
        jax.block_until_ready(rs)
        t_hi = _time.perf_counter() - t0
        del rs
        t0 = _time.perf_counter()
        rs = [fn(*dev_in, *dev_zero) for _ in range(iters_lo)]
        jax.block_until_ready(rs)
        t_lo = _time.perf_counter() - t0
        del rs
        per_exec = (t_hi - t_lo) / (iters - iters_lo)
        print(
            f"  t({iters})={t_hi*1e3:.2f}ms t({iters_lo})={t_lo*1e3:.2f}ms "
            f"per_exec={per_exec*1e6:.1f}us"
        )
        if best is None or per_exec < best:
            best = per_exec
    return best * 1e9


# revision 12
# speedup vs baseline: 1.7835x; 1.3428x over previous
"""FAGCNConv Trainium2 kernel (8 NeuronCores, destination-sharded edges). v3

Algorithm (matches reference up to fp rounding):
    s2b = x @ w2 + b                      (per destination node, local slice)
    sr_e = x[row_e] @ w1                  (per edge, fused DVE mul-reduce on gathered rows)
    sc_e = s2b[col_e]                     (per edge, fused one-hot dot vs broadcast s2 block)
    p_e  = exp(tanh(sr_e + sc_e))         (tanh bounds scores, so softmax max-shift is unneeded)
    acc[v], segsum[v] = sum_{e->v} p_e * x[row_e],  sum_{e->v} p_e   (one-hot matmuls into PSUM)
    out[v] = (1-EPS) * acc[v]/segsum[v] + EPS * x[v]

v3 changes (why): HW probing showed the kernel is bound by dma_gather's
per-descriptor latency (~8.6ns/desc aggregate at 512B single-queue).
  - x table gathered in bf16 (256B elements: ~6.1ns/desc, -30%)
  - gathers striped across 4 SWDGE queues (~1.3x from latency hiding)
  - per-block tile counts (max over cores) instead of global max (-8% descs)
  - one-hot + propagation matmuls in bf16 (PE fp32 4cyc/row -> bf16 1cyc/row)

Sharding: core c owns destinations [c*6250, (c+1)*6250), 49 blocks of 128 dst.
Host sorts edges by (block, row>=32768) and pads each block's lo/hi sections to
the max count over the 8 cores so the SPMD program is identical across cores.
"""

import os
import sys

sys.path.insert(0, "/opt/trn_rl_repo")

import numpy as np
import ml_dtypes

BF16 = np.dtype(ml_dtypes.bfloat16)

N_NODES = 50000
C = 128
EPS = 0.1
NCORES = 8
NLOC = N_NODES // NCORES          # 6250
NBLK = (NLOC + 127) // 128        # 49 (48 full, last has 106 dst)
P = 128
HALF = 32768                      # int16 index limit for dma_gather
DUMMY_COLREL = 200.0
A2_CHUNK = 512
NQ = 4                            # SWDGE queues to stripe gathers over
CE = 256                          # gather element width in bf16: [x(128)|1.0|pad]


def _wrap_idx16(lst):
    """dma_gather index layout: [128, N/16] int16; idx i at [i%16, i//16],
    replicated across the 8 groups of 16 partitions."""
    n = len(lst)
    assert n % 128 == 0
    a16 = np.zeros((16, n // 16), dtype=np.int16)
    a16[np.arange(n) % 16, np.arange(n) // 16] = lst
    return np.tile(a16, (8, 1))


def _prep_shards(edge_index: np.ndarray):
    row_g = edge_index[0].astype(np.int64)
    col_g = edge_index[1].astype(np.int64)
    core_of = col_g // NLOC

    per_core = []
    cnt = np.zeros((NCORES, NBLK, 2), dtype=np.int64)
    for c in range(NCORES):
        m = core_of == c
        r = row_g[m]
        cl = col_g[m] - c * NLOC
        blk = cl // P
        hi = (r >= HALF).astype(np.int64)
        key = blk * 2 + hi
        counts = np.bincount(key, minlength=NBLK * 2)
        cnt[c] = counts.reshape(NBLK, 2)
        per_core.append((r, cl, blk, hi, key, counts))

    cmax = cnt.max(axis=0)                      # [NBLK, 2]
    TBLs = ((cmax[:, 0] + P - 1) // P).astype(int).tolist()
    TBHs = ((cmax[:, 1] + P - 1) // P).astype(int).tolist()
    TBs = [a + b for a, b in zip(TBLs, TBHs)]
    TILES = int(sum(TBs))
    # tile-offset of each block's section
    toff = np.concatenate([[0], np.cumsum(TBs)]).astype(int)

    shards = []
    for c in range(NCORES):
        r, cl, blk, hi, key, counts = per_core[c]
        idx_slot = np.zeros(TILES * P, dtype=np.int64)
        colrel_slot = np.full(TILES * P, DUMMY_COLREL, dtype=np.float32)

        order = np.argsort(key, kind="stable")
        starts = np.zeros(NBLK * 2, dtype=np.int64)
        starts[1:] = np.cumsum(counts)[:-1]
        pos_in_sec = np.arange(len(order)) - starts[key[order]]
        ro, clo, blko, hio = r[order], cl[order], blk[order], hi[order]
        sec_base = (toff[blko] + hio * np.asarray(TBLs)[blko]) * P
        slot = sec_base + pos_in_sec
        idx_slot[slot] = ro - hio * HALF
        colrel_slot[slot] = (clo - blko * P).astype(np.float32)

        idx16_cols = []
        for b in range(NBLK):
            base = toff[b] * P
            mid = base + TBLs[b] * P
            end = base + TBs[b] * P
            idx16_cols.append(_wrap_idx16(idx_slot[base:mid]))
            idx16_cols.append(_wrap_idx16(idx_slot[mid:end]))
        idx16 = np.concatenate(idx16_cols, axis=1)  # [128, TILES*8]
        colrel_T = np.ascontiguousarray(
            colrel_slot.reshape(TILES, P).T
        )  # [128, TILES]
        shards.append(dict(idx16=idx16, colrel_T=colrel_T))
    return tuple(TBLs), tuple(TBHs), shards


def _build_nc(TBLs, TBHs):
    import concourse.bacc as bacc
    import concourse.bass as bass
    import concourse.mybir as mybir
    from concourse.tile import TileContext

    f32 = mybir.dt.float32
    bf16 = mybir.dt.bfloat16
    i16 = mybir.dt.int16
    TBs = [a + b for a, b in zip(TBLs, TBHs)]
    TILES = int(sum(TBs))
    toff = [0]
    for t in TBs:
        toff.append(toff[-1] + t)
    NLOC_PAD = NBLK * P  # 6272

    nc = bacc.Bacc("TRN2", target_bir_lowering=False, num_swdge_queues=NQ)

    xlo_d = nc.dram_tensor("xlo", [HALF, CE], bf16, kind="ExternalInput")
    xhi_d = nc.dram_tensor("xhi", [N_NODES - HALF, CE], bf16, kind="ExternalInput")
    xloc_d = nc.dram_tensor("xloc", [NLOC, C], f32, kind="ExternalInput")
    xlocT_d = nc.dram_tensor("xlocT", [P, NLOC], f32, kind="ExternalInput")
    idx16_d = nc.dram_tensor("idx16", [P, TILES * 8], i16, kind="ExternalInput")
    colrel_d = nc.dram_tensor("colrel", [P, TILES], f32, kind="ExternalInput")
    gw_d = nc.dram_tensor("gate_w", [2 * C, 1], f32, kind="ExternalInput")
    gb_d = nc.dram_tensor("gate_b", [1], f32, kind="ExternalInput")
    iota_d = nc.dram_tensor("iotaf", [P, P], f32, kind="ExternalInput")
    w1b_d = nc.dram_tensor("w1b", [P, P], f32, kind="ExternalInput")
    out_d = nc.dram_tensor("out", [NLOC, C], f32, kind="ExternalOutput")

    s2b_d = nc.dram_tensor("s2b_scratch", [1, NLOC_PAD], f32)

    with TileContext(nc) as tc:
        with (
            tc.tile_pool(name="const", bufs=1) as cpool,
            tc.tile_pool(name="phA", bufs=3) as apool,
            tc.tile_pool(name="phA_ps", bufs=2, space="PSUM") as apsum,
            tc.tile_pool(name="ybuf", bufs=4) as ypool,
            tc.tile_pool(name="blk", bufs=4) as bpool,
            tc.tile_pool(name="small", bufs=6) as spool,
            tc.tile_pool(name="oh", bufs=22) as ohpool,
            tc.tile_pool(name="acc_ps", bufs=3, space="PSUM") as bpsum,
        ):
            iotaf = cpool.tile([P, P], f32)
            nc.sync.dma_start(iotaf[:], iota_d[:])
            iotab = cpool.tile([P, P], bf16)
            nc.vector.tensor_scalar(
                iotab[:], iotaf[:], 1.0, None, op0=mybir.AluOpType.mult
            )
            w1b = cpool.tile([P, P], f32)
            nc.sync.dma_start(w1b[:], w1b_d[:])
            w2 = cpool.tile([P, 1], f32)
            nc.sync.dma_start(w2[:], gw_d[C : 2 * C, 0:1])
            btile = cpool.tile([1, 1], f32)
            nc.sync.dma_start(btile[:], gb_d[:, None])
            ones_col = cpool.tile([P, 1], bf16)
            nc.vector.memset(ones_col[:], 1.0)
            zpad = cpool.tile([1, NLOC_PAD - NLOC], f32)
            nc.vector.memset(zpad[:], 0.0)
            nc.sync.dma_start(s2b_d[0:1, NLOC:NLOC_PAD], zpad[:])
            s2all = cpool.tile([P, NLOC_PAD], f32)

            # ---- Phase A: s2b[v] = xloc[v] @ w2 + b (local nodes) ----
            nck = (NLOC + A2_CHUNK - 1) // A2_CHUNK
            for k in range(nck):
                a = k * A2_CHUNK
                n = min(A2_CHUNK, NLOC - a)
                xck = apool.tile([P, A2_CHUNK], f32, tag="xck")
                nc.sync.dma_start(xck[:, :n], xlocT_d[:, a : a + n])
                ps = apsum.tile([1, A2_CHUNK], f32, tag="s2ps")
                nc.tensor.matmul(
                    out=ps[:, :n], lhsT=w2[:], rhs=xck[:, :n], start=True, stop=True
                )
                s2sb = apool.tile([1, A2_CHUNK], f32, tag="s2sb")
                nc.scalar.activation(
                    s2sb[:, :n],
                    ps[:, :n],
                    mybir.ActivationFunctionType.Identity,
                    bias=btile[:],
                    scale=1.0,
                )
                nc.sync.dma_start(s2b_d[0:1, a : a + n], s2sb[:, :n])

            # one broadcast of the whole s2 row into SBUF (replaces 49
            # per-block 64KB broadcast DMAs)
            nc.sync.dma_start(
                s2all[:], s2b_d[0:1, :].to_broadcast((P, NLOC_PAD))
            )

            # ---- Phase B ----
            nblk_run = int(os.environ.get("KERNEL_NBLK", NBLK))
            skips = set(os.environ.get("KERNEL_SKIP", "").split(","))
            qctr = 0
            for b in range(nblk_run):
                TBL, TBH = TBLs[b], TBHs[b]
                TB = TBL + TBH
                nd = min(P, NLOC - b * P)
                t0 = toff[b]

                colrel_blk = spool.tile([P, TB], f32, tag="colrel")
                nc.sync.dma_start(colrel_blk[:], colrel_d[:, t0 : t0 + TB])
                idxlo = spool.tile([P, TBL * 8], i16, tag="idxlo")
                nc.sync.dma_start(
                    idxlo[:], idx16_d[:, t0 * 8 : t0 * 8 + TBL * 8]
                )
                idxhi = spool.tile([P, TBH * 8], i16, tag="idxhi")
                nc.sync.dma_start(
                    idxhi[:], idx16_d[:, t0 * 8 + TBL * 8 : (t0 + TB) * 8]
                )
                s2bc = s2all[:, b * P : (b + 1) * P]

                Y_lo = ypool.tile([P, TBL * CE], bf16, tag="Ylo")
                Y_hi = ypool.tile([P, TBH * CE], bf16, tag="Yhi")
                if "gather" in skips:
                    nc.vector.memset(Y_lo[:], 0.5)
                    nc.vector.memset(Y_hi[:], 0.5)
                else:
                    nc.gpsimd.dma_gather(
                        Y_lo[:].rearrange("p (t c) -> p t c", c=CE),
                        xlo_d[:],
                        idxlo[:],
                        TBL * P,
                        TBL * P,
                        CE,
                        single_packet=False,
                        queue_num=qctr % NQ,
                    )
                    qctr += 1
                    nc.gpsimd.dma_gather(
                        Y_hi[:].rearrange("p (t c) -> p t c", c=CE),
                        xhi_d[:],
                        idxhi[:],
                        TBH * P,
                        TBH * P,
                        CE,
                        single_packet=False,
                        queue_num=qctr % NQ,
                    )
                    qctr += 1

                def _ytile(t):
                    if t < TBL:
                        return Y_lo[:, t * CE : t * CE + C + 1]
                    return Y_hi[:, (t - TBL) * CE : (t - TBL) * CE + C + 1]

                sr_blk = spool.tile([P, TB], f32, tag="sr")
                sc_blk = spool.tile([P, TB], f32, tag="sc")
                if "stt" in skips:
                    nc.vector.memset(sr_blk[:], 0.1)
                    nc.vector.memset(sc_blk[:], 0.1)
                for t in range([0, TB]["stt" not in skips]):
                    scr1 = ohpool.tile([P, P], bf16, tag="scr1")
                    nc.vector.scalar_tensor_tensor(
                        out=scr1[:],
                        in0=_ytile(t)[:, :C],
                        scalar=1.0,
                        in1=w1b[:],
                        op0=mybir.AluOpType.mult,
                        op1=mybir.AluOpType.mult,
                        accum_out=sr_blk[:, t : t + 1],
                    )
                    scr2 = ohpool.tile([P, P], bf16, tag="scr2")
                    nc.vector.scalar_tensor_tensor(
                        out=scr2[:],
                        in0=iotaf[:],
                        scalar=colrel_blk[:, t : t + 1],
                        in1=s2bc,
                        op0=mybir.AluOpType.is_equal,
                        op1=mybir.AluOpType.mult,
                        accum_out=sc_blk[:, t : t + 1],
                    )

                u_blk = spool.tile([P, TB], f32, tag="u")
                nc.vector.tensor_tensor(
                    out=u_blk[:],
                    in0=sr_blk[:],
                    in1=sc_blk[:],
                    op=mybir.AluOpType.add,
                )
                th_blk = spool.tile([P, TB], f32, tag="th")
                nc.scalar.activation(
                    th_blk[:], u_blk[:], mybir.ActivationFunctionType.Tanh
                )
                p_blk = spool.tile([P, TB], f32, tag="p")
                nc.scalar.activation(
                    p_blk[:], th_blk[:], mybir.ActivationFunctionType.Exp
                )

                # col 128 of each gathered element is a host-written 1.0, so a
                # single [P, C+1] matmul accumulates propagation AND segsum.
                acc_x = bpsum.tile([P, C + 1], f32, tag="accx")
                for t in range([0, TB]["mm" not in skips]):
                    onehot = ohpool.tile([P, P], bf16, tag="onehot")
                    nc.vector.tensor_scalar(
                        onehot[:],
                        iotab[:],
                        colrel_blk[:, t : t + 1],
                        p_blk[:, t : t + 1],
                        op0=mybir.AluOpType.is_equal,
                        op1=mybir.AluOpType.mult,
                    )
                    nc.tensor.matmul(
                        out=acc_x[:],
                        lhsT=onehot[:],
                        rhs=_ytile(t),
                        start=(t == 0),
                        stop=(t == TB - 1),
                    )

                if "mm" in skips:
                    nc.tensor.matmul(out=acc_x[:, :C], lhsT=iotab[:], rhs=iotab[:], start=True, stop=False)
                    nc.tensor.matmul(out=acc_x[:, C : C + 1], lhsT=iotab[:], rhs=ones_col[:], start=False, stop=True)
                segsum = spool.tile([P, 1], f32, tag="segsum")
                nc.vector.tensor_scalar(
                    segsum[:], acc_x[:, C : C + 1], 1e-30, None, op0=mybir.AluOpType.add
                )
                inv = spool.tile([P, 1], f32, tag="inv")
                nc.vector.reciprocal(inv[:], segsum[:])
                inv9 = spool.tile([P, 1], f32, tag="inv9")
                nc.scalar.mul(inv9[:], inv[:], 1.0 - EPS)

                xblk = bpool.tile([P, C], f32, tag="xblk")
                nc.sync.dma_start(xblk[:nd, :], xloc_d[b * P : b * P + nd, :])
                o1 = bpool.tile([P, C], f32, tag="o1")
                nc.vector.tensor_scalar(
                    o1[:], acc_x[:, :C], inv9[:], None, op0=mybir.AluOpType.mult
                )
                oblk = bpool.tile([P, C], f32, tag="oblk")
                nc.vector.scalar_tensor_tensor(
                    oblk[:nd, :],
                    xblk[:nd, :],
                    EPS,
                    o1[:nd, :],
                    op0=mybir.AluOpType.mult,
                    op1=mybir.AluOpType.add,
                )
                nc.sync.dma_start(out_d[b * P : b * P + nd, :], oblk[:nd, :])

    nc.finalize()
    return nc


_CACHE = {}


def _get_nc(TBLs, TBHs):
    key = (TBLs, TBHs)
    if key not in _CACHE:
        _CACHE[key] = _build_nc(TBLs, TBHs)
    return _CACHE[key]


def _make_in_maps(x, edge_index, gate_w, gate_b):
    TBLs, TBHs, shards = _prep_shards(edge_index)
    iotaf = np.broadcast_to(np.arange(P, dtype=np.float32)[None, :], (P, P)).copy()
    w1b = np.broadcast_to(gate_w[:C, 0][None, :], (P, C)).copy()
    xaug = np.zeros((N_NODES, CE), dtype=BF16)
    xaug[:, :C] = x.astype(BF16)
    xaug[:, C] = np.float32(1.0)
    xlo = np.ascontiguousarray(xaug[:HALF])
    xhi = np.ascontiguousarray(xaug[HALF:])
    in_maps = []
    for c in range(NCORES):
        xloc = np.ascontiguousarray(x[c * NLOC : (c + 1) * NLOC])
        in_maps.append(
            {
                "xlo": xlo,
                "xhi": xhi,
                "xloc": xloc,
                "xlocT": np.ascontiguousarray(xloc.T),
                "idx16": shards[c]["idx16"],
                "colrel": shards[c]["colrel_T"],
                "gate_w": gate_w,
                "gate_b": gate_b,
                "iotaf": iotaf,
                "w1b": w1b,
            }
        )
    return TBLs, TBHs, in_maps


def kernel(x, edge_index, gate_w, gate_b):
    from concourse.bass_utils import run_bass_kernel_spmd

    x = np.asarray(x, dtype=np.float32)
    edge_index = np.asarray(edge_index, dtype=np.int32)
    gate_w = np.asarray(gate_w, dtype=np.float32)
    gate_b = np.asarray(gate_b, dtype=np.float32)

    TBLs, TBHs, in_maps = _make_in_maps(x, edge_index, gate_w, gate_b)
    nc = _get_nc(TBLs, TBHs)

    res = run_bass_kernel_spmd(nc, in_maps, core_ids=list(range(NCORES)))
    out = np.concatenate([res.results[c]["out"] for c in range(NCORES)], axis=0)
    return out


def time_kernel(inputs, iters=32, iters_lo=2, reps=4):
    """Estimate per-execution HW time: async-dispatch M executions of one jitted
    single-exec program (device executions serialize per core); per-exec time =
    (T(M_hi) - T(M_lo)) / (M_hi - M_lo), min over reps."""
    import time as _time

    import jax
    import concourse.mybir as mybir
    from concourse import bass2jax as b2j

    x = np.asarray(inputs["x"], dtype=np.float32)
    edge_index = np.asarray(inputs["edge_index"], dtype=np.int32)
    gate_w = np.asarray(inputs["gate_w"], dtype=np.float32)
    gate_b = np.asarray(inputs["gate_b"], dtype=np.float32)

    TBLs, TBHs, in_maps = _make_in_maps(x, edge_index, gate_w, gate_b)
    nc = _get_nc(TBLs, TBHs)
    b2j.install_neuronx_cc_hook()

    partition_name = nc.partition_id_tensor.name if nc.partition_id_tensor else None
    in_names, out_names, out_avals, zero_outs = [], [], [], []
    for alloc in nc.m.functions[0].allocations:
        if not isinstance(alloc, mybir.MemoryLocationSet):
            continue
        name = alloc.memorylocations[0].name
        if alloc.kind == "ExternalInput":
            if name != partition_name:
                in_names.append(name)
        elif alloc.kind == "ExternalOutput":
            shape = tuple(alloc.tensor_shape)
            dtype = mybir.dt.np(alloc.dtype)
            out_names.append(name)
            out_avals.append(jax.core.ShapedArray(shape, dtype))
            zero_outs.append(np.zeros(shape, dtype))
    n_params = len(in_names)
    all_in_names = in_names + out_names

    def _body(*args):
        operands = list(args)
        if partition_name is not None:
            operands.append(b2j.partition_id_tensor())
        return tuple(
            b2j._bass_exec_p.bind(
                *operands,
                out_avals=tuple(out_avals),
                in_names=tuple(
                    all_in_names + ([partition_name] if partition_name else [])
                ),
                out_names=tuple(out_names),
                lowering_input_output_aliases=(),
                sim_require_finite=True,
                sim_require_nnan=True,
                nc=nc,
            )
        )

    devices = jax.devices()[:NCORES]
    mesh = b2j.Mesh(np.asarray(devices), ("core",))
    in_specs = (b2j.PartitionSpec("core",),) * (n_params + len(out_names))
    out_specs = (b2j.PartitionSpec("core",),) * len(out_names)
    fn = jax.jit(
        b2j.shard_map(
            _body, mesh=mesh, in_specs=in_specs, out_specs=out_specs, check_rep=False
        ),
        keep_unused=True,
    )

    per_core = [[np.asarray(m[name]) for name in in_names] for m in in_maps]
    concat_in = [
        np.concatenate([per_core[c][i] for c in range(NCORES)], axis=0)
        for i in range(n_params)
    ]
    concat_zeros = [
        np.zeros((NCORES * z.shape[0], *z.shape[1:]), z.dtype) for z in zero_outs
    ]

    from jax.sharding import NamedSharding

    sh = NamedSharding(mesh, b2j.PartitionSpec("core"))
    dev_in = [jax.device_put(a, sh) for a in concat_in]
    dev_zero = [jax.device_put(a, sh) for a in concat_zeros]

    jax.block_until_ready(fn(*dev_in, *dev_zero))
    jax.block_until_ready(fn(*dev_in, *dev_zero))

    best = None
    for _ in range(reps):
        t0 = _time.perf_counter()
        rs = [fn(*dev_in, *dev_zero) for _ in range(iters)]
        jax.block_until_ready(rs)
        t_hi = _time.perf_counter() - t0
        del rs
        t0 = _time.perf_counter()
        rs = [fn(*dev_in, *dev_zero) for _ in range(iters_lo)]
        jax.block_until_ready(rs)
        t_lo = _time.perf_counter() - t0
        del rs
        per_exec = (t_hi - t_lo) / (iters - iters_lo)
        print(
            f"  t({iters})={t_hi*1e3:.2f}ms t({iters_lo})={t_lo*1e3:.2f}ms "
            f"per_exec={per_exec*1e6:.1f}us"
        )
        if best is None or per_exec < best:
            best = per_exec
    return best * 1e9
